# revision 1
# baseline (speedup 1.0000x reference)
"""MoE (16 routed experts, top-2, + shared expert) on 8 TRN2 NeuronCores.

Strategy (expert-parallel per the sharding hint):
  Launch A (SPMD, data-parallel over tokens): each core takes a 2048-token
    slice, computes router logits/softmax/top-2 combine weights on-device
    (fp32 matmul for exact-ish selection) and the shared-expert SwiGLU FFN
    (fp32r matmuls = bf16-speed).  Outputs: comb (2048x16), shared y^T.
  Host: reads comb, builds per-expert token index lists, gathers token
    vectors into dense per-expert batches (the "all-to-all dispatch").
  Launch B (SPMD, expert-parallel): core c owns experts 2c and 2c+1; runs
    the SwiGLU FFN on each expert's gathered batch, scaling rows by the
    combine weight on-device.  Outputs: weighted y^T per expert.
  Host: scatter-adds expert outputs + shared outputs into the full result
    (the "combine").

All activations travel transposed (feature-major, token-minor) so every
matmul operand loads with natural DMA strides and zero on-device transposes.
"""

import math

import numpy as np

# model dims (fixed for this problem)
E, TOPK, C, I = 16, 2, 768, 1536
B, T = 8, 2048
NCORE = 8
NTOK = B * T           # 16384
TPC = NTOK // NCORE    # 2048 tokens per core
CK = C // 128          # 6 contraction chunks for C
IK = I // 128          # 12 chunks for I
NBLK = 512             # token block = PE moving-dim per matmul

TRACE = False          # set True (from a driver) to capture NTFF timing
LAST = {}              # timing info from the most recent kernel() call

_progs = {}            # compiled program cache


def _enable_axon_ntff_profiling():
    import sys
    import types

    if "antenv.axon_hooks" not in sys.modules:
        mod = types.ModuleType("antenv.axon_hooks")
        mod._hook = None
        mod.set_axon_ntff_profile_hook = lambda h: setattr(mod, "_hook", h)
        mod.get_axon_ntff_profile_hook = lambda: mod._hook
        sys.modules["antenv.axon_hooks"] = mod
    from antenv.axon_hooks import set_axon_ntff_profile_hook  # type: ignore
    from trn_agent_boot.trn_boot import _ntff_profile_via_ctypes

    set_axon_ntff_profile_hook(_ntff_profile_via_ctypes("/opt/axon/libaxon_pjrt.so"))
    import concourse.bass_utils as bu

    bu.upload_artifacts = lambda tmpdir: f"file://{tmpdir}"


def _blocks(m):
    """Split m tokens into PE-friendly blocks (<=512 each)."""
    out = []
    n0 = 0
    while n0 < m:
        nb = min(NBLK, m - n0)
        out.append((n0, nb))
        n0 += nb
    return out


def _emit_ffn_block(nc, pools, x_all, wg_sb, wu_sb, wd_sb, scale_sb, y_ap, n0, nblk):
    """One token-block of SwiGLU FFN in transposed layout.

    x_all: SBUF [128, CK, NBLK] (c-major, token-minor) for this block
    wg_sb/wu_sb: SBUF [128, CK, I]; wd_sb: SBUF [128, IK, C]
    scale_sb: SBUF [128, cap] per-token combine weight (or None)
    y_ap: DRAM (C, M) output, written at columns [n0, n0+nblk)
    """
    import concourse.mybir as mybir

    f32 = mybir.dt.float32
    f32r = mybir.dt.float32r
    hpool, gpool, ypool, pgu, pd = (
        pools["h"],
        pools["g"],
        pools["y"],
        pools["pgu"],
        pools["pd"],
    )

    h_all = hpool.tile([128, IK, NBLK], f32r, tag="h_all")
    for ik in range(IK):
        psg = pgu.tile([128, NBLK], f32, tag="psg")
        psu = pgu.tile([128, NBLK], f32, tag="psu")
        for ck in range(CK):
            nc.tensor.matmul(
                psg[:, :nblk],
                lhsT=wg_sb[:, ck, ik * 128 : (ik + 1) * 128],
                rhs=x_all[:, ck, :nblk],
                start=(ck == 0),
                stop=(ck == CK - 1),
            )
        for ck in range(CK):
            nc.tensor.matmul(
                psu[:, :nblk],
                lhsT=wu_sb[:, ck, ik * 128 : (ik + 1) * 128],
                rhs=x_all[:, ck, :nblk],
                start=(ck == 0),
                stop=(ck == CK - 1),
            )
        ga = gpool.tile([128, NBLK], f32, tag="ga")
        nc.scalar.activation(
            ga[:, :nblk], psg[:, :nblk], mybir.ActivationFunctionType.Silu
        )
        nc.vector.tensor_mul(h_all[:, ik, :nblk], ga[:, :nblk], psu[:, :nblk])

    for ck in range(CK):
        psd = pd.tile([128, NBLK], f32, tag="psd")
        for ik in range(IK):
            nc.tensor.matmul(
                psd[:, :nblk],
                lhsT=wd_sb[:, ik, ck * 128 : (ck + 1) * 128],
                rhs=h_all[:, ik, :nblk],
                start=(ik == 0),
                stop=(ik == IK - 1),
            )
        yb = ypool.tile([128, NBLK], f32, tag="yb")
        if scale_sb is None:
            nc.vector.tensor_copy(yb[:, :nblk], psd[:, :nblk])
        else:
            nc.vector.tensor_mul(
                yb[:, :nblk], psd[:, :nblk], scale_sb[:, n0 : n0 + nblk]
            )
        nc.sync.dma_start(
            out=y_ap[ck * 128 : (ck + 1) * 128, n0 : n0 + nblk], in_=yb[:, :nblk]
        )


def _build_launch_a():
    """Router + shared expert, one 2048-token slice per core."""
    from contextlib import ExitStack

    import concourse.tile as tile
    from concourse import bacc, mybir

    f32 = mybir.dt.float32
    AX = mybir.AxisListType.X
    OP = mybir.AluOpType

    nc = bacc.Bacc("TRN2", target_bir_lowering=False, debug=False)
    f32r = mybir.dt.float32r
    xt_ap = nc.dram_tensor("xt", [C, TPC], f32, kind="ExternalInput").ap()
    wgate_ap = nc.dram_tensor("wgate", [C, E], f32, kind="ExternalInput").ap()
    biasb_ap = nc.dram_tensor("biasb", [128, E], f32, kind="ExternalInput").ap()
    swg_ap = nc.dram_tensor("swg", [C, I], f32r, kind="ExternalInput").ap()
    swu_ap = nc.dram_tensor("swu", [C, I], f32r, kind="ExternalInput").ap()
    swd_ap = nc.dram_tensor("swd", [I, C], f32r, kind="ExternalInput").ap()
    comb_ap = nc.dram_tensor("comb", [TPC, E], f32, kind="ExternalOutput").ap()
    yst_ap = nc.dram_tensor("yst", [C, TPC], f32, kind="ExternalOutput").ap()

    with tile.TileContext(nc) as tc, ExitStack() as ctx:
        wpool = ctx.enter_context(tc.tile_pool(name="weights", bufs=1))
        xpool = ctx.enter_context(tc.tile_pool(name="xp", bufs=2))
        hpool = ctx.enter_context(tc.tile_pool(name="hp", bufs=1))
        gpool = ctx.enter_context(tc.tile_pool(name="gp", bufs=2))
        ypool = ctx.enter_context(tc.tile_pool(name="yp", bufs=3))
        rpool = ctx.enter_context(tc.tile_pool(name="rp", bufs=2))
        pgu = ctx.enter_context(tc.tile_pool(name="pgu", bufs=2, space="PSUM"))
        pd = ctx.enter_context(tc.tile_pool(name="pd", bufs=2, space="PSUM"))
        pr = ctx.enter_context(tc.tile_pool(name="pr", bufs=2, space="PSUM"))
        xrpool = ctx.enter_context(tc.tile_pool(name="xr", bufs=1))
        pools = {"h": hpool, "g": gpool, "y": ypool, "pgu": pgu, "pd": pd}

        wgate_sb = wpool.tile([128, CK, E], f32, tag="wgate")
        swg_sb = wpool.tile([128, CK, I], f32r, tag="swg")
        swu_sb = wpool.tile([128, CK, I], f32r, tag="swu")
        swd_sb = wpool.tile([128, IK, C], f32r, tag="swd")
        bias_sb = wpool.tile([128, E], f32, tag="bias")
        for ck in range(CK):
            nc.sync.dma_start(
                out=swg_sb[:, ck, :], in_=swg_ap[ck * 128 : (ck + 1) * 128, :]
            )
        for ck in range(CK):
            nc.sync.dma_start(
                out=wgate_sb[:, ck, :], in_=wgate_ap[ck * 128 : (ck + 1) * 128, :]
            )
        nc.sync.dma_start(out=bias_sb[:], in_=biasb_ap[:])
        for ck in range(CK):
            nc.sync.dma_start(
                out=swu_sb[:, ck, :], in_=swu_ap[ck * 128 : (ck + 1) * 128, :]
            )
        for ik in range(IK):
            nc.sync.dma_start(
                out=swd_sb[:, ik, :], in_=swd_ap[ik * 128 : (ik + 1) * 128, :]
            )

        for n in range(TPC // NBLK):
            x32 = xpool.tile([128, CK, NBLK], f32, tag="x32")
            for ck in range(CK):
                nc.sync.dma_start(
                    out=x32[:, ck, :],
                    in_=xt_ap[ck * 128 : (ck + 1) * 128, n * NBLK : (n + 1) * NBLK],
                )
            x_all = xrpool.tile([128, CK, NBLK], f32r, tag="x_all")
            nc.vector.tensor_copy(x_all[:], x32[:])
            # router: tokens as PSUM partitions, 4 chunks of 128 per block
            for q in range(NBLK // 128):
                t0 = q * 128
                psl = pr.tile([128, E], f32, tag="psl")
                for ck in range(CK):
                    nc.tensor.matmul(
                        psl[:],
                        lhsT=x32[:, ck, t0 : t0 + 128],
                        rhs=wgate_sb[:, ck, :],
                        start=(ck == 0),
                        stop=(ck == CK - 1),
                    )
                lg = rpool.tile([128, E], f32, tag="lg")
                nc.vector.tensor_add(lg[:], psl[:], bias_sb[:])
                m1 = rpool.tile([128, 1], f32, tag="m1")
                nc.vector.reduce_max(m1[:], lg[:], axis=AX)
                nm1 = rpool.tile([128, 1], f32, tag="nm1")
                nc.vector.tensor_scalar_mul(nm1[:], m1[:], -1.0)
                ex = rpool.tile([128, E], f32, tag="ex")
                nc.scalar.activation(
                    ex[:], lg[:], mybir.ActivationFunctionType.Exp, bias=nm1[:]
                )
                msk1 = rpool.tile([128, E], f32, tag="msk1")
                nc.vector.tensor_scalar(msk1[:], lg[:], m1[:], None, op0=OP.is_equal)
                pen = rpool.tile([128, E], f32, tag="pen")
                nc.vector.tensor_scalar_mul(pen[:], msk1[:], 1e30)
                lm = rpool.tile([128, E], f32, tag="lm")
                nc.vector.tensor_sub(lm[:], lg[:], pen[:])
                m2 = rpool.tile([128, 1], f32, tag="m2")
                nc.vector.reduce_max(m2[:], lm[:], axis=AX)
                ge = rpool.tile([128, E], f32, tag="ge")
                nc.vector.tensor_scalar(ge[:], lg[:], m2[:], None, op0=OP.is_ge)
                we = rpool.tile([128, E], f32, tag="we")
                nc.vector.tensor_mul(we[:], ex[:], ge[:])
                sm = rpool.tile([128, 1], f32, tag="sm")
                nc.vector.reduce_sum(sm[:], we[:], axis=AX)
                rs = rpool.tile([128, 1], f32, tag="rs")
                nc.vector.reciprocal(rs[:], sm[:])
                cmb = rpool.tile([128, E], f32, tag="cmb")
                nc.vector.tensor_scalar(cmb[:], we[:], rs[:], None, op0=OP.mult)
                nc.sync.dma_start(
                    out=comb_ap[n * NBLK + t0 : n * NBLK + t0 + 128, :], in_=cmb[:]
                )
            # shared expert FFN on this block
            _emit_ffn_block(
                nc, pools, x_all, swg_sb, swu_sb, swd_sb, None, yst_ap, n * NBLK, NBLK
            )

    nc.compile()
    return nc


def _build_launch_b(cap):
    """Two routed experts per core on dense gathered batches of size cap."""
    from contextlib import ExitStack

    import concourse.tile as tile
    from concourse import bacc, mybir

    f32 = mybir.dt.float32
    f32r = mybir.dt.float32r

    nc = bacc.Bacc("TRN2", target_bir_lowering=False, debug=False)
    aps = {}
    for s in ("a", "b"):
        aps[f"x{s}"] = nc.dram_tensor(f"x{s}t", [C, cap], f32r, kind="ExternalInput").ap()
        aps[f"wg{s}"] = nc.dram_tensor(f"wg{s}", [C, I], f32r, kind="ExternalInput").ap()
        aps[f"wu{s}"] = nc.dram_tensor(f"wu{s}", [C, I], f32r, kind="ExternalInput").ap()
        aps[f"wd{s}"] = nc.dram_tensor(f"wd{s}", [I, C], f32r, kind="ExternalInput").ap()
        aps[f"sc{s}"] = nc.dram_tensor(f"sc{s}", [128, cap], f32, kind="ExternalInput").ap()
        aps[f"y{s}"] = nc.dram_tensor(f"y{s}t", [C, cap], f32, kind="ExternalOutput").ap()

    with tile.TileContext(nc) as tc, ExitStack() as ctx:
        wpool = ctx.enter_context(tc.tile_pool(name="weights", bufs=1))
        xpool = ctx.enter_context(tc.tile_pool(name="xp", bufs=2))
        hpool = ctx.enter_context(tc.tile_pool(name="hp", bufs=1))
        gpool = ctx.enter_context(tc.tile_pool(name="gp", bufs=2))
        ypool = ctx.enter_context(tc.tile_pool(name="yp", bufs=3))
        spool = ctx.enter_context(tc.tile_pool(name="sp", bufs=1))
        pgu = ctx.enter_context(tc.tile_pool(name="pgu", bufs=2, space="PSUM"))
        pd = ctx.enter_context(tc.tile_pool(name="pd", bufs=2, space="PSUM"))
        pools = {"h": hpool, "g": gpool, "y": ypool, "pgu": pgu, "pd": pd}

        for s in ("a", "b"):
            wg_sb = wpool.tile([128, CK, I], f32r, tag="wg")
            wu_sb = wpool.tile([128, CK, I], f32r, tag="wu")
            wd_sb = wpool.tile([128, IK, C], f32r, tag="wd")
            sc_sb = spool.tile([128, cap], f32, tag="sc")
            for ck in range(CK):
                nc.sync.dma_start(
                    out=wg_sb[:, ck, :], in_=aps[f"wg{s}"][ck * 128 : (ck + 1) * 128, :]
                )
            for ck in range(CK):
                nc.sync.dma_start(
                    out=wu_sb[:, ck, :], in_=aps[f"wu{s}"][ck * 128 : (ck + 1) * 128, :]
                )
            for ik in range(IK):
                nc.sync.dma_start(
                    out=wd_sb[:, ik, :], in_=aps[f"wd{s}"][ik * 128 : (ik + 1) * 128, :]
                )
            nc.sync.dma_start(out=sc_sb[:], in_=aps[f"sc{s}"][:])
            for n0, nblk in _blocks(cap):
                x_all = xpool.tile([128, CK, NBLK], f32r, tag="x_all")
                for ck in range(CK):
                    nc.sync.dma_start(
                        out=x_all[:, ck, :nblk],
                        in_=aps[f"x{s}"][ck * 128 : (ck + 1) * 128, n0 : n0 + nblk],
                    )
                _emit_ffn_block(
                    nc, pools, x_all, wg_sb, wu_sb, wd_sb, sc_sb, aps[f"y{s}"], n0, nblk
                )

    nc.compile()
    return nc


def _run(nc, in_maps, tag):
    from concourse.bass_utils import run_bass_kernel_spmd

    if TRACE:
        _enable_axon_ntff_profiling()
        res = run_bass_kernel_spmd(nc, in_maps, list(range(NCORE)), trace=True)
        LAST[f"{tag}_ns"] = res.exec_time_ns
        if res.instructions_and_trace is not None:
            LAST[f"{tag}_trace"] = res.instructions_and_trace[1]
    else:
        res = run_bass_kernel_spmd(nc, in_maps, list(range(NCORE)), trace=False)
    return res.results


def kernel(x, w_gate, expert_bias, wg, wu, wd, swg, swu, swd):
    LAST.clear()
    xf = np.ascontiguousarray(np.asarray(x, np.float32).reshape(NTOK, C))
    w_gate = np.ascontiguousarray(np.asarray(w_gate, np.float32))
    expert_bias = np.asarray(expert_bias, np.float32)
    wg = np.asarray(wg, np.float32)
    wu = np.asarray(wu, np.float32)
    wd = np.asarray(wd, np.float32)
    swg = np.ascontiguousarray(np.asarray(swg, np.float32))
    swu = np.ascontiguousarray(np.asarray(swu, np.float32))
    swd = np.ascontiguousarray(np.asarray(swd, np.float32))

    xt_full = np.ascontiguousarray(xf.T)  # (C, NTOK)
    bias_b = np.ascontiguousarray(np.broadcast_to(expert_bias, (128, E)))

    # ---- launch A: router + shared expert
    if "A" not in _progs:
        _progs["A"] = _build_launch_a()
    in_maps = []
    for c in range(NCORE):
        in_maps.append(
            {
                "xt": np.ascontiguousarray(xt_full[:, c * TPC : (c + 1) * TPC]),
                "wgate": w_gate,
                "biasb": bias_b,
                "swg": swg,
                "swu": swu,
                "swd": swd,
            }
        )
    res_a = _run(_progs["A"], in_maps, "launchA")

    comb = np.concatenate([res_a[c]["comb"] for c in range(NCORE)], axis=0)

    # ---- host routing: per-expert index lists + weights
    idxs, wts = [], []
    for e in range(E):
        ii = np.nonzero(comb[:, e] > 0.0)[0]
        idxs.append(ii)
        wts.append(comb[ii, e].astype(np.float32))
    max_cnt = max(len(ii) for ii in idxs)
    cap = max(NBLK, ((max_cnt + 127) // 128) * 128)

    # ---- launch B: routed experts (2 per core)
    key = ("B", cap)
    if key not in _progs:
        _progs[key] = _build_launch_b(cap)
    in_maps_b = []
    for c in range(NCORE):
        m = {}
        for s, e in (("a", 2 * c), ("b", 2 * c + 1)):
            ii, ww = idxs[e], wts[e]
            xt = np.zeros((C, cap), np.float32)
            xt[:, : len(ii)] = xf[ii].T
            sc = np.zeros((128, cap), np.float32)
            sc[:, : len(ii)] = ww[None, :]
            m[f"x{s}t"] = xt
            m[f"sc{s}"] = sc
            m[f"wg{s}"] = np.ascontiguousarray(wg[e])
            m[f"wu{s}"] = np.ascontiguousarray(wu[e])
            m[f"wd{s}"] = np.ascontiguousarray(wd[e])
        in_maps_b.append(m)
    res_b = _run(_progs[key], in_maps_b, "launchB")

    # ---- host combine: shared + scattered weighted expert outputs
    out = np.empty((NTOK, C), np.float32)
    for c in range(NCORE):
        out[c * TPC : (c + 1) * TPC] = res_a[c]["yst"].T
    for e in range(E):
        c, s = e // 2, ("a", "b")[e % 2]
        y = res_b[c][f"y{s}t"]  # (C, cap), already comb-weighted
        out[idxs[e]] += y[:, : len(idxs[e])].T

    if TRACE:
        LAST["total_ns"] = sum(
            v for k, v in LAST.items() if isinstance(v, int) and k.endswith("_ns")
        )
    return out.reshape(B, T, C)



# revision 3
# speedup vs baseline: 1.3455x; 1.3455x over previous
"""MoE (16 routed experts, top-2, + shared expert) on 8 TRN2 NeuronCores.

Strategy (expert-parallel per the sharding hint):
  Host computes the router (softmax + top-2 + renormalize, exactly
  mirroring the reference math) -- this is the dispatch computation that
  decides the expert-parallel sharding.  Tokens are gathered into dense
  per-expert batches (the all-to-all dispatch), experts are paired
  (largest with smallest count) so the 8 cores get balanced slots.

  One SPMD launch does all the FLOPs: each core runs
    - the shared-expert SwiGLU FFN on its 2048-token slice (data-parallel)
    - two routed experts' SwiGLU FFNs on their gathered token batches.
  All matmul operands are bf16 (full PE rate, half the DMA of fp32),
  accumulation in fp32 PSUM.  Activations travel transposed
  (feature-major, token-minor) so every matmul loads with natural DMA
  strides and zero on-device transposes.

  Host combine: scatter-add  comb_weight * expert_out  plus the shared
  output into the full result (the all-to-all combine).
"""

import numpy as np
import ml_dtypes

BF16 = ml_dtypes.bfloat16

# model dims (fixed for this problem)
E, TOPK, C, I = 16, 2, 768, 1536
B, T = 8, 2048
NCORE = 8
NTOK = B * T           # 16384
TPC = NTOK // NCORE    # 2048 tokens per core (shared-expert slice)
CK = C // 128          # 6 contraction chunks for C
IK = I // 128          # 12 chunks for I
NBLK = 512             # token block = PE moving-dim per matmul

TRACE = False          # set True (from a driver) to capture NTFF timing
LAST = {}              # timing info from the most recent kernel() call

_progs = {}            # compiled program cache


def _enable_axon_ntff_profiling():
    import sys
    import types

    if "antenv.axon_hooks" not in sys.modules:
        mod = types.ModuleType("antenv.axon_hooks")
        mod._hook = None
        mod.set_axon_ntff_profile_hook = lambda h: setattr(mod, "_hook", h)
        mod.get_axon_ntff_profile_hook = lambda: mod._hook
        sys.modules["antenv.axon_hooks"] = mod
    from antenv.axon_hooks import set_axon_ntff_profile_hook  # type: ignore
    from trn_agent_boot.trn_boot import _ntff_profile_via_ctypes

    set_axon_ntff_profile_hook(_ntff_profile_via_ctypes("/opt/axon/libaxon_pjrt.so"))
    import concourse.bass_utils as bu

    bu.upload_artifacts = lambda tmpdir: f"file://{tmpdir}"


def _blocks(m):
    """Split m tokens into PE-friendly blocks (<=512 each)."""
    out = []
    n0 = 0
    while n0 < m:
        nb = min(NBLK, m - n0)
        out.append((n0, nb))
        n0 += nb
    return out


def _emit_ffn_block(nc, pools, x_all, wg_sb, wu_sb, wd_sb, y_ap, n0, nblk):
    """One token-block of SwiGLU FFN in transposed layout (all bf16).

    x_all: SBUF [128, CK, NBLK] bf16 (c-major, token-minor) for this block
    wg_sb/wu_sb: SBUF [128, CK, I] bf16; wd_sb: SBUF [128, IK, C] bf16
    y_ap: DRAM (C, M) bf16 output, written at columns [n0, n0+nblk)
    """
    import concourse.mybir as mybir

    f32 = mybir.dt.float32
    bf = mybir.dt.bfloat16
    hpool, gpool, ypool, pgu, pd = (
        pools["h"],
        pools["g"],
        pools["y"],
        pools["pgu"],
        pools["pd"],
    )

    h_all = hpool.tile([128, IK, NBLK], bf, tag="h_all")
    for ik in range(IK):
        psg = pgu.tile([128, NBLK], f32, tag="psg")
        psu = pgu.tile([128, NBLK], f32, tag="psu")
        for ck in range(CK):
            nc.tensor.matmul(
                psg[:, :nblk],
                lhsT=wg_sb[:, ck, ik * 128 : (ik + 1) * 128],
                rhs=x_all[:, ck, :nblk],
                start=(ck == 0),
                stop=(ck == CK - 1),
            )
        for ck in range(CK):
            nc.tensor.matmul(
                psu[:, :nblk],
                lhsT=wu_sb[:, ck, ik * 128 : (ik + 1) * 128],
                rhs=x_all[:, ck, :nblk],
                start=(ck == 0),
                stop=(ck == CK - 1),
            )
        ga = gpool.tile([128, NBLK], f32, tag="ga")
        nc.scalar.activation(
            ga[:, :nblk], psg[:, :nblk], mybir.ActivationFunctionType.Silu
        )
        nc.vector.tensor_mul(h_all[:, ik, :nblk], ga[:, :nblk], psu[:, :nblk])

    for ck in range(CK):
        psd = pd.tile([128, NBLK], f32, tag="psd")
        for ik in range(IK):
            nc.tensor.matmul(
                psd[:, :nblk],
                lhsT=wd_sb[:, ik, ck * 128 : (ck + 1) * 128],
                rhs=h_all[:, ik, :nblk],
                start=(ik == 0),
                stop=(ik == IK - 1),
            )
        yb = ypool.tile([128, NBLK], bf, tag="yb")
        nc.vector.tensor_copy(yb[:, :nblk], psd[:, :nblk])
        nc.sync.dma_start(
            out=y_ap[ck * 128 : (ck + 1) * 128, n0 : n0 + nblk], in_=yb[:, :nblk]
        )


def _build(cap_a, cap_b):
    """Single launch: shared expert on the 2048-token slice + 2 routed
    experts on gathered batches of size cap_a / cap_b."""
    from contextlib import ExitStack

    import concourse.tile as tile
    from concourse import bacc, mybir

    bf = mybir.dt.bfloat16

    nc = bacc.Bacc("TRN2", target_bir_lowering=False, debug=False)
    slots = []
    for s, cap in (("s", TPC), ("a", cap_a), ("b", cap_b)):
        x_ap = nc.dram_tensor(f"x{s}", [C, cap], bf, kind="ExternalInput").ap()
        wg_ap = nc.dram_tensor(f"wg{s}", [C, I], bf, kind="ExternalInput").ap()
        wu_ap = nc.dram_tensor(f"wu{s}", [C, I], bf, kind="ExternalInput").ap()
        wd_ap = nc.dram_tensor(f"wd{s}", [I, C], bf, kind="ExternalInput").ap()
        y_ap = nc.dram_tensor(f"y{s}", [C, cap], bf, kind="ExternalOutput").ap()
        slots.append((x_ap, wg_ap, wu_ap, wd_ap, y_ap, cap))

    with tile.TileContext(nc) as tc, ExitStack() as ctx:
        wpool = ctx.enter_context(tc.tile_pool(name="weights", bufs=2))
        xpool = ctx.enter_context(tc.tile_pool(name="xp", bufs=2))
        hpool = ctx.enter_context(tc.tile_pool(name="hp", bufs=2))
        gpool = ctx.enter_context(tc.tile_pool(name="gp", bufs=2))
        ypool = ctx.enter_context(tc.tile_pool(name="yp", bufs=3))
        pgu = ctx.enter_context(tc.tile_pool(name="pgu", bufs=2, space="PSUM"))
        pd = ctx.enter_context(tc.tile_pool(name="pd", bufs=2, space="PSUM"))
        pools = {"h": hpool, "g": gpool, "y": ypool, "pgu": pgu, "pd": pd}

        for x_ap, wg_ap, wu_ap, wd_ap, y_ap, cap in slots:
            wg_sb = wpool.tile([128, CK, I], bf, tag="wg")
            wu_sb = wpool.tile([128, CK, I], bf, tag="wu")
            wd_sb = wpool.tile([128, IK, C], bf, tag="wd")
            # first block's x + gate/up weights first so the PE starts ASAP;
            # the 6+6+6 descriptors spread across parallel DMA queues.
            n0_0, nblk_0 = _blocks(cap)[0]
            x0 = xpool.tile([128, CK, NBLK], bf, tag="x_all")
            for ck in range(CK):
                nc.sync.dma_start(out=x0[:, ck, :nblk_0], in_=x_ap[ck * 128 : (ck + 1) * 128, :nblk_0])
            for ck in range(CK):
                nc.sync.dma_start(out=wg_sb[:, ck, :], in_=wg_ap[ck * 128 : (ck + 1) * 128, :])
            for ck in range(CK):
                nc.sync.dma_start(out=wu_sb[:, ck, :], in_=wu_ap[ck * 128 : (ck + 1) * 128, :])
            for ik in range(IK):
                nc.sync.dma_start(out=wd_sb[:, ik, :], in_=wd_ap[ik * 128 : (ik + 1) * 128, :])

            for bi, (n0, nblk) in enumerate(_blocks(cap)):
                if bi == 0:
                    x_all = x0
                else:
                    x_all = xpool.tile([128, CK, NBLK], bf, tag="x_all")
                    for ck in range(CK):
                        nc.sync.dma_start(
                            out=x_all[:, ck, :nblk],
                            in_=x_ap[ck * 128 : (ck + 1) * 128, n0 : n0 + nblk],
                        )
                _emit_ffn_block(nc, pools, x_all, wg_sb, wu_sb, wd_sb, y_ap, n0, nblk)

    nc.compile()
    return nc


def _run(nc, in_maps, tag):
    from concourse.bass_utils import run_bass_kernel_spmd

    if TRACE:
        _enable_axon_ntff_profiling()
        res = run_bass_kernel_spmd(nc, in_maps, list(range(NCORE)), trace=True)
        LAST[f"{tag}_ns"] = res.exec_time_ns
        if res.instructions_and_trace is not None:
            LAST[f"{tag}_trace"] = res.instructions_and_trace[1]
    else:
        res = run_bass_kernel_spmd(nc, in_maps, list(range(NCORE)), trace=False)
    return res.results


def kernel(x, w_gate, expert_bias, wg, wu, wd, swg, swu, swd):
    LAST.clear()
    xf = np.ascontiguousarray(np.asarray(x, np.float32).reshape(NTOK, C))
    w_gate = np.asarray(w_gate, np.float32)
    expert_bias = np.asarray(expert_bias, np.float32)
    wg = np.asarray(wg, np.float32)
    wu = np.asarray(wu, np.float32)
    wd = np.asarray(wd, np.float32)

    # ---- host router: exact replica of the reference math (fp32)
    logits = xf @ w_gate + expert_bias
    m = logits.max(axis=1, keepdims=True)
    p = np.exp(logits - m)
    p /= p.sum(axis=1, keepdims=True)
    order = np.argsort(-p, axis=1, kind="stable")[:, :TOPK]  # == lax.top_k order
    tp = np.take_along_axis(p, order, 1)
    tp = tp / tp.sum(axis=1, keepdims=True)

    idxs, wts = [], []
    for e in range(E):
        sel = np.nonzero(order == e)
        idxs.append(sel[0])
        wts.append(tp[sel].astype(np.float32))
    cnt = np.array([len(ii) for ii in idxs])

    # ---- pair experts: core i gets (i-th largest, i-th smallest)
    dsc = np.argsort(-cnt, kind="stable")
    slot_a = [int(dsc[i]) for i in range(NCORE)]
    slot_b = [int(dsc[E - 1 - i]) for i in range(NCORE)]
    rnd = 64
    cap_a = max(256, (int(cnt[slot_a[0]]) + rnd - 1) // rnd * rnd)
    cap_b = max(256, (int(cnt[dsc[NCORE]]) + rnd - 1) // rnd * rnd)

    # ---- dense transposed bf16 operands
    x_bf = xf.astype(BF16)
    xt_full = np.ascontiguousarray(x_bf.T)  # (C, NTOK) bf16

    def wset(g, u, d):
        return (
            np.ascontiguousarray(g.astype(BF16)),
            np.ascontiguousarray(u.astype(BF16)),
            np.ascontiguousarray(d.astype(BF16)),
        )

    def gather(e, cap):
        xt = np.zeros((C, cap), BF16)
        ii = idxs[e]
        xt[:, : len(ii)] = np.ascontiguousarray(x_bf[ii].T)
        return xt

    key = (cap_a, cap_b)
    if key not in _progs:
        _progs[key] = _build(cap_a, cap_b)

    sg, su, sd = wset(np.asarray(swg, np.float32), np.asarray(swu, np.float32),
                      np.asarray(swd, np.float32))
    in_maps = []
    for c in range(NCORE):
        ea, eb = slot_a[c], slot_b[c]
        ag, au, ad = wset(wg[ea], wu[ea], wd[ea])
        bg, bu, bd = wset(wg[eb], wu[eb], wd[eb])
        in_maps.append(
            {
                "xs": np.ascontiguousarray(xt_full[:, c * TPC : (c + 1) * TPC]),
                "xa": gather(ea, cap_a),
                "xb": gather(eb, cap_b),
                "wgs": sg, "wus": su, "wds": sd,
                "wga": ag, "wua": au, "wda": ad,
                "wgb": bg, "wub": bu, "wdb": bd,
            }
        )
    res = _run(_progs[key], in_maps, "launchC")

    # ---- host combine: shared + weighted scatter-add of expert outputs
    out = np.empty((NTOK, C), np.float32)
    for c in range(NCORE):
        out[c * TPC : (c + 1) * TPC] = res[c]["ys"].T.astype(np.float32)
    for c in range(NCORE):
        for e, nm in ((slot_a[c], "ya"), (slot_b[c], "yb")):
            ii = idxs[e]
            if len(ii) == 0:
                continue
            y = res[c][nm][:, : len(ii)].T.astype(np.float32)
            out[ii] += wts[e][:, None] * y

    if TRACE:
        LAST["total_ns"] = sum(
            v for k, v in LAST.items() if isinstance(v, int) and k.endswith("_ns")
        )
    return out.reshape(B, T, C)


# revision 5
# speedup vs baseline: 1.3561x; 1.0079x over previous
"""MoE (16 routed experts, top-2, + shared expert) on 8 TRN2 NeuronCores.

Strategy (expert-parallel per the sharding hint):
  Host computes the router (softmax + top-2 + renormalize, exactly
  mirroring the reference math) -- this is the dispatch computation that
  decides the expert-parallel sharding.  Tokens are gathered into dense
  per-expert batches (the all-to-all dispatch), experts are paired
  (largest with smallest count) so the 8 cores get balanced slots.

  One SPMD launch does all the FLOPs: each core runs
    - the shared-expert SwiGLU FFN on its 2048-token slice (data-parallel)
    - two routed experts' SwiGLU FFNs on their gathered token batches.
  All matmul operands are bf16 (full PE rate, half the DMA of fp32),
  accumulation in fp32 PSUM.  Activations travel transposed
  (feature-major, token-minor) so every matmul loads with natural DMA
  strides and zero on-device transposes.

  Host combine: scatter-add  comb_weight * expert_out  plus the shared
  output into the full result (the all-to-all combine).
"""

import numpy as np
import ml_dtypes

BF16 = ml_dtypes.bfloat16

# model dims (fixed for this problem)
E, TOPK, C, I = 16, 2, 768, 1536
B, T = 8, 2048
NCORE = 8
NTOK = B * T           # 16384
TPC = NTOK // NCORE    # 2048 tokens per core (shared-expert slice)
CK = C // 128          # 6 contraction chunks for C
IK = I // 128          # 12 chunks for I
NBLK = 512             # token block = PE moving-dim per matmul

TRACE = False          # set True (from a driver) to capture NTFF timing
LAST = {}              # timing info from the most recent kernel() call

_progs = {}            # compiled program cache


def _enable_axon_ntff_profiling():
    import sys
    import types

    if "antenv.axon_hooks" not in sys.modules:
        mod = types.ModuleType("antenv.axon_hooks")
        mod._hook = None
        mod.set_axon_ntff_profile_hook = lambda h: setattr(mod, "_hook", h)
        mod.get_axon_ntff_profile_hook = lambda: mod._hook
        sys.modules["antenv.axon_hooks"] = mod
    from antenv.axon_hooks import set_axon_ntff_profile_hook  # type: ignore
    from trn_agent_boot.trn_boot import _ntff_profile_via_ctypes

    set_axon_ntff_profile_hook(_ntff_profile_via_ctypes("/opt/axon/libaxon_pjrt.so"))
    import concourse.bass_utils as bu

    bu.upload_artifacts = lambda tmpdir: f"file://{tmpdir}"


def _blocks(m):
    """Split m tokens into PE-friendly blocks (<=512 each)."""
    out = []
    n0 = 0
    while n0 < m:
        nb = min(NBLK, m - n0)
        out.append((n0, nb))
        n0 += nb
    return out


def _emit_ffn_block(nc, pools, x_all, wg_sb, wu_sb, wd_sb, y_ap, n0, nblk):
    """One token-block of SwiGLU FFN in transposed layout (all bf16).

    x_all: SBUF [128, CK, NBLK] bf16 (c-major, token-minor) for this block
    wg_sb/wu_sb: SBUF [128, CK, I] bf16; wd_sb: SBUF [128, IK, C] bf16
    y_ap: DRAM (C, M) bf16 output, written at columns [n0, n0+nblk)
    """
    import concourse.mybir as mybir

    f32 = mybir.dt.float32
    bf = mybir.dt.bfloat16
    hpool, gpool, ypool, pgu, pd = (
        pools["h"],
        pools["g"],
        pools["y"],
        pools["pgu"],
        pools["pd"],
    )

    h_all = hpool.tile([128, IK, NBLK], bf, tag="h_all")
    for ik in range(IK):
        psg = pgu.tile([128, NBLK], f32, tag="psg")
        psu = pgu.tile([128, NBLK], f32, tag="psu")
        for ck in range(CK):
            nc.tensor.matmul(
                psg[:, :nblk],
                lhsT=wg_sb[:, ck, ik * 128 : (ik + 1) * 128],
                rhs=x_all[:, ck, :nblk],
                start=(ck == 0),
                stop=(ck == CK - 1),
            )
        for ck in range(CK):
            nc.tensor.matmul(
                psu[:, :nblk],
                lhsT=wu_sb[:, ck, ik * 128 : (ik + 1) * 128],
                rhs=x_all[:, ck, :nblk],
                start=(ck == 0),
                stop=(ck == CK - 1),
            )
        ga = gpool.tile([128, NBLK], f32, tag="ga")
        nc.scalar.activation(
            ga[:, :nblk], psg[:, :nblk], mybir.ActivationFunctionType.Silu
        )
        nc.vector.tensor_mul(h_all[:, ik, :nblk], ga[:, :nblk], psu[:, :nblk])

    for ck in range(CK):
        psd = pd.tile([128, NBLK], f32, tag="psd")
        for ik in range(IK):
            nc.tensor.matmul(
                psd[:, :nblk],
                lhsT=wd_sb[:, ik, ck * 128 : (ck + 1) * 128],
                rhs=h_all[:, ik, :nblk],
                start=(ik == 0),
                stop=(ik == IK - 1),
            )
        yb = ypool.tile([128, NBLK], bf, tag="yb")
        nc.vector.tensor_copy(yb[:, :nblk], psd[:, :nblk])
        nc.sync.dma_start(
            out=y_ap[ck * 128 : (ck + 1) * 128, n0 : n0 + nblk], in_=yb[:, :nblk]
        )


def _build(cap_a, cap_b):
    """Single launch: shared expert on the 2048-token slice + 2 routed
    experts on gathered batches of size cap_a / cap_b."""
    from contextlib import ExitStack

    import concourse.tile as tile
    from concourse import bacc, mybir

    bf = mybir.dt.bfloat16

    nc = bacc.Bacc("TRN2", target_bir_lowering=False, debug=False)
    slots = []
    for s, cap in (("s", TPC), ("a", cap_a), ("b", cap_b)):
        x_ap = nc.dram_tensor(f"x{s}", [C, cap], bf, kind="ExternalInput").ap()
        wg_ap = nc.dram_tensor(f"wg{s}", [C, I], bf, kind="ExternalInput").ap()
        wu_ap = nc.dram_tensor(f"wu{s}", [C, I], bf, kind="ExternalInput").ap()
        wd_ap = nc.dram_tensor(f"wd{s}", [I, C], bf, kind="ExternalInput").ap()
        y_ap = nc.dram_tensor(f"y{s}", [C, cap], bf, kind="ExternalOutput").ap()
        slots.append((x_ap, wg_ap, wu_ap, wd_ap, y_ap, cap))

    with tile.TileContext(nc) as tc, ExitStack() as ctx:
        wpool = ctx.enter_context(tc.tile_pool(name="weights", bufs=2))
        xpool = ctx.enter_context(tc.tile_pool(name="xp", bufs=2))
        hpool = ctx.enter_context(tc.tile_pool(name="hp", bufs=2))
        gpool = ctx.enter_context(tc.tile_pool(name="gp", bufs=2))
        ypool = ctx.enter_context(tc.tile_pool(name="yp", bufs=3))
        pgu = ctx.enter_context(tc.tile_pool(name="pgu", bufs=2, space="PSUM"))
        pd = ctx.enter_context(tc.tile_pool(name="pd", bufs=2, space="PSUM"))
        pools = {"h": hpool, "g": gpool, "y": ypool, "pgu": pgu, "pd": pd}

        for si, (x_ap, wg_ap, wu_ap, wd_ap, y_ap, cap) in enumerate(slots):
            wg_sb = wpool.tile([128, CK, I], bf, tag="wg")
            wu_sb = wpool.tile([128, CK, I], bf, tag="wu")
            wd_sb = wpool.tile([128, IK, C], bf, tag="wd")
            # first block's x + gate/up weights first so the PE starts ASAP;
            # descriptors spread across parallel DMA queues.  For the first
            # slot, chunk wg/wu along I in consumption order (the first psg
            # needs only the first 128-col chunk of every ck) so the PE can
            # start after ~100KB instead of after the full 4.7MB.
            n0_0, nblk_0 = _blocks(cap)[0]
            x0 = xpool.tile([128, CK, NBLK], bf, tag="x_all")
            for ck in range(CK):
                nc.sync.dma_start(out=x0[:, ck, :nblk_0], in_=x_ap[ck * 128 : (ck + 1) * 128, :nblk_0])
            nq = 4 if si == 0 else 1
            qw = I // nq
            for q in range(nq):
                for w_sb, w_ap in ((wg_sb, wg_ap), (wu_sb, wu_ap)):
                    for ck in range(CK):
                        nc.sync.dma_start(
                            out=w_sb[:, ck, q * qw : (q + 1) * qw],
                            in_=w_ap[ck * 128 : (ck + 1) * 128, q * qw : (q + 1) * qw],
                        )
            for ik in range(IK):
                nc.sync.dma_start(out=wd_sb[:, ik, :], in_=wd_ap[ik * 128 : (ik + 1) * 128, :])

            for bi, (n0, nblk) in enumerate(_blocks(cap)):
                if bi == 0:
                    x_all = x0
                else:
                    x_all = xpool.tile([128, CK, NBLK], bf, tag="x_all")
                    for ck in range(CK):
                        nc.sync.dma_start(
                            out=x_all[:, ck, :nblk],
                            in_=x_ap[ck * 128 : (ck + 1) * 128, n0 : n0 + nblk],
                        )
                _emit_ffn_block(nc, pools, x_all, wg_sb, wu_sb, wd_sb, y_ap, n0, nblk)

    nc.compile()
    return nc


def _run(nc, in_maps, tag):
    from concourse.bass_utils import run_bass_kernel_spmd

    if TRACE:
        _enable_axon_ntff_profiling()
        res = run_bass_kernel_spmd(nc, in_maps, list(range(NCORE)), trace=True)
        LAST[f"{tag}_ns"] = res.exec_time_ns
        if res.instructions_and_trace is not None:
            LAST[f"{tag}_trace"] = res.instructions_and_trace[1]
    else:
        res = run_bass_kernel_spmd(nc, in_maps, list(range(NCORE)), trace=False)
    return res.results


def kernel(x, w_gate, expert_bias, wg, wu, wd, swg, swu, swd):
    LAST.clear()
    xf = np.ascontiguousarray(np.asarray(x, np.float32).reshape(NTOK, C))
    w_gate = np.asarray(w_gate, np.float32)
    expert_bias = np.asarray(expert_bias, np.float32)
    wg = np.asarray(wg, np.float32)
    wu = np.asarray(wu, np.float32)
    wd = np.asarray(wd, np.float32)

    # ---- host router: exact replica of the reference math (fp32)
    logits = xf @ w_gate + expert_bias
    m = logits.max(axis=1, keepdims=True)
    p = np.exp(logits - m)
    p /= p.sum(axis=1, keepdims=True)
    order = np.argsort(-p, axis=1, kind="stable")[:, :TOPK]  # == lax.top_k order
    tp = np.take_along_axis(p, order, 1)
    tp = tp / tp.sum(axis=1, keepdims=True)

    idxs, wts = [], []
    for e in range(E):
        sel = np.nonzero(order == e)
        idxs.append(sel[0])
        wts.append(tp[sel].astype(np.float32))
    cnt = np.array([len(ii) for ii in idxs])

    # ---- pair experts: core i gets (i-th largest, i-th smallest)
    dsc = np.argsort(-cnt, kind="stable")
    slot_a = [int(dsc[i]) for i in range(NCORE)]
    slot_b = [int(dsc[E - 1 - i]) for i in range(NCORE)]
    rnd = 16
    cap_a = max(256, (int(cnt[slot_a[0]]) + rnd - 1) // rnd * rnd)
    cap_b = max(256, (int(cnt[dsc[NCORE]]) + rnd - 1) // rnd * rnd)

    # ---- dense transposed bf16 operands
    x_bf = xf.astype(BF16)
    xt_full = np.ascontiguousarray(x_bf.T)  # (C, NTOK) bf16

    def wset(g, u, d):
        return (
            np.ascontiguousarray(g.astype(BF16)),
            np.ascontiguousarray(u.astype(BF16)),
            np.ascontiguousarray(d.astype(BF16)),
        )

    def gather(e, cap):
        xt = np.zeros((C, cap), BF16)
        ii = idxs[e]
        xt[:, : len(ii)] = np.ascontiguousarray(x_bf[ii].T)
        return xt

    key = (cap_a, cap_b)
    if key not in _progs:
        _progs[key] = _build(cap_a, cap_b)

    sg, su, sd = wset(np.asarray(swg, np.float32), np.asarray(swu, np.float32),
                      np.asarray(swd, np.float32))
    in_maps = []
    for c in range(NCORE):
        ea, eb = slot_a[c], slot_b[c]
        ag, au, ad = wset(wg[ea], wu[ea], wd[ea])
        bg, bu, bd = wset(wg[eb], wu[eb], wd[eb])
        in_maps.append(
            {
                "xs": np.ascontiguousarray(xt_full[:, c * TPC : (c + 1) * TPC]),
                "xa": gather(ea, cap_a),
                "xb": gather(eb, cap_b),
                "wgs": sg, "wus": su, "wds": sd,
                "wga": ag, "wua": au, "wda": ad,
                "wgb": bg, "wub": bu, "wdb": bd,
            }
        )
    res = _run(_progs[key], in_maps, "launchC")

    # ---- host combine: shared + weighted scatter-add of expert outputs
    out = np.empty((NTOK, C), np.float32)
    for c in range(NCORE):
        out[c * TPC : (c + 1) * TPC] = res[c]["ys"].T.astype(np.float32)
    for c in range(NCORE):
        for e, nm in ((slot_a[c], "ya"), (slot_b[c], "yb")):
            ii = idxs[e]
            if len(ii) == 0:
                continue
            y = res[c][nm][:, : len(ii)].T.astype(np.float32)
            out[ii] += wts[e][:, None] * y

    if TRACE:
        LAST["total_ns"] = sum(
            v for k, v in LAST.items() if isinstance(v, int) and k.endswith("_ns")
        )
    return out.reshape(B, T, C)
